# revision 15
# baseline (speedup 1.0000x reference)
"""CharCNN Trainium2 Bass kernel (v2).

Self-contained: hardcodes shapes/sharding for the nn_CharCNN problem:
  X_scan [64, 512, 50] int64, emb [257, 16], 7 conv1d branches (k=1..7,
  480 total channels) + max-pool + leaky_relu, 2 highway layers, proj to 768.

Strategy (pure data parallelism over 8 cores, 4096 words each):
  host:  stride-1 im2col rows: per word, 50 patch slots of 128 rows
         (7 chars x 16 dims = rows 0..111, validity-indicator rows 112..118:
         -1e30 where slot invalid for filter k, row 112+(7-k) carries weight
         1.0 in the conv lhsT).  Laid out [blk, 128, 50*256] bf16 so device
         loads are plain contiguous DMAs (no transpose DMAs).
  conv:  per (chunk i, 128-word block): 13 matmuls (12x N=512, 1x N=256),
         rhs streams w-contiguous columns (slot-major groups of 4).
         Invalid slots come out ~-1e30 so pooling needs no masking.
  pool:  split across engines: DVE tensor_reduce on PSUM tile T1 (16 slots),
         ACT copies T2, T3, T4 -> SBUF bf16, DVE tensor_tensor max tree
         (2x mode on bf16) folds everything to [CH, 128].
  highway/proj: feature-major matmuls, K-chunks {96,128,128,128}
  out:   feature-major [768, W] per core; host transposes/concats.
"""

import functools
import os
import sys

import numpy as np

for _p in ("/opt/trn_rl_repo",):
    if _p not in sys.path and os.path.isdir(_p):
        sys.path.insert(0, _p)

import ml_dtypes  # noqa: E402

BF16 = ml_dtypes.bfloat16

# ---------------- model dims ----------------
B, T = 64, 512
MAX_CHARS = 50
CHAR_D = 16
D = 768
FILTERS = [(1, 32), (2, 32), (3, 32), (4, 64), (5, 64), (6, 128), (7, 128)]
NF = 480
N_CORES = 8
W_TOTAL = B * T  # 32768
W = W_TOTAL // N_CORES  # 4096

NSLOT = 50        # patch slots per word (stride-1)
PAD_IDX = 257     # zero row in padded emb table (258 rows)
NEG_BIG = -1.0e30

WBLK = 128        # words per conv/pool unit
BLK = 256         # words per im2col DMA block
N_BLK = W // BLK  # 16
ROWCOL = NSLOT * BLK  # 12800 columns per P-block row

# channel chunking (natural filter order): chunk i covers CH_OFF[i]..+CH_SZ[i]
CH_OFF = [0, 96, 224, 352]
CH_SZ = [96, 128, 128, 128]
CHUNK_KS = [
    [(1, 0, 32), (2, 32, 64), (3, 64, 96)],
    [(4, 0, 64), (5, 64, 128)],
    [(6, 0, 128)],
    [(7, 0, 128)],
]
K_BASE = {1: 0, 2: 32, 3: 64, 4: 96, 5: 160, 6: 224, 7: 352}

# highway K-chunks == channel chunks; M-chunks same
HW_SZ = CH_SZ
HW_OFF = CH_OFF
PROJ_M = [(i * 128, 128) for i in range(6)]   # 768 = 6 x 128

WT = 512          # words per highway/proj tile

F_CONV = 8        # PE keep-warm filler matmuls per conv unit
F_HW = 1          # fillers per highway (hwi, i) step


# ---------------- host-side parameter prep ----------------

def _prep_conv_lhst(inputs):
    """[128, 480] bf16: 4 chunk blocks; rows 0..111 weights, 112..118 indicator."""
    A = np.zeros((128, NF), np.float32)
    for i in range(4):
        for (k, lo, hi) in CHUNK_KS[i]:
            wk = np.asarray(inputs[f"conv{k}_w"], np.float32)  # [O,16,k]
            for oloc in range(lo, hi):
                col = CH_OFF[i] + oloc
                o_f = CH_OFF[i] + oloc - K_BASE[k]
                for d in range(k):
                    A[16 * d:16 * d + 16, col] = wk[o_f, :, d]
                A[112 + (7 - k), col] = 1.0
    return A.astype(BF16)


def _prep_hw_lhst(w):
    """hw w [960,480] -> [128, 3840] bf16; blocks (which, i, kc)."""
    w = np.asarray(w, np.float32)
    cols = []
    for which in (0, 1):  # 0=nonlin rows 0:480, 1=gate rows 480:960
        for i in range(4):
            for kc in range(4):
                blk = w[480 * which + HW_OFF[i]:480 * which + HW_OFF[i] + HW_SZ[i],
                        HW_OFF[kc]:HW_OFF[kc] + HW_SZ[kc]]
                A = np.zeros((128, HW_SZ[i]), np.float32)
                A[:HW_SZ[kc], :] = blk.T
                cols.append(A)
    return np.concatenate(cols, axis=1).astype(BF16)


def _prep_proj_lhst(w):
    w = np.asarray(w, np.float32)
    cols = []
    for (mo, ms) in PROJ_M:
        for kc in range(4):
            blk = w[mo:mo + ms, HW_OFF[kc]:HW_OFF[kc] + HW_SZ[kc]]
            A = np.zeros((128, ms), np.float32)
            A[:HW_SZ[kc], :] = blk.T
            cols.append(A)
    return np.concatenate(cols, axis=1).astype(BF16)


def _prep_biases(inputs):
    """[128, 26] fp32 columns: conv(4) hw1nl(4) hw1g(4) hw2nl(4) hw2g(4) proj(6)."""
    conv_b = np.concatenate([np.asarray(inputs[f"conv{k}_b"], np.float32)
                             for k, _ in FILTERS])
    cols = np.zeros((128, 26), np.float32)
    c = 0
    for i in range(4):
        cols[:CH_SZ[i], c] = conv_b[CH_OFF[i]:CH_OFF[i] + CH_SZ[i]]; c += 1
    for name in ("hw1_b", "hw2_b"):
        b = np.asarray(inputs[name], np.float32)
        for half in (0, 480):
            for i in range(4):
                cols[:CH_SZ[i], c] = b[half + CH_OFF[i]:half + CH_OFF[i] + CH_SZ[i]]
                c += 1
    pb = np.asarray(inputs["proj_b"], np.float32)
    for (mo, ms) in PROJ_M:
        cols[:ms, c] = pb[mo:mo + ms]; c += 1
    assert c == 26
    return cols

BIAS_CONV = 0
BIAS_HW = {1: {"nl": 4, "g": 8}, 2: {"nl": 12, "g": 16}}
BIAS_PROJ = 20


def _prep_pdram(Xc, emb_bf):
    """Build the im2col array [N_BLK*128, ROWCOL] bf16 for one core.

    Xc: [W, 50] int32 char indices; emb_bf: [258, 16] bf16 (row 257 = 0).
    P[p, t, w]: p in 0..111 -> char (t + p//16) emb dim (p%16); 112..118
    indicator rows; 119..127 zero.
    """
    Xp = np.full((W, MAX_CHARS + 7), PAD_IDX, np.int32)
    Xp[:, :MAX_CHARS] = Xc
    E = emb_bf[Xp]                                    # [W, 57, 16] bf16
    P = np.zeros((128, NSLOT, W), BF16)
    for t in range(NSLOT):
        # [W, 7, 16] -> [W, 112] -> [112, W]
        P[0:112, t, :] = E[:, t:t + 7, :].reshape(W, 112).T
    for j in range(7):
        thr = 44 + j
        if thr < NSLOT:
            P[112 + j, thr:, :] = BF16(NEG_BIG)
    # [128, 50, W] -> [N_BLK, 128, 50, BLK] -> [N_BLK*128, 50*BLK]
    P = P.reshape(128, NSLOT, N_BLK, BLK).transpose(2, 0, 1, 3)
    return np.ascontiguousarray(P).reshape(N_BLK * 128, ROWCOL)


# ---------------- device program ----------------

@functools.lru_cache(maxsize=2)
def _build(W_arg):
    assert W_arg == W
    from concourse import bass, mybir
    from concourse.tile import TileContext
    from contextlib import ExitStack

    dt = mybir.dt
    f32, bf16 = dt.float32, dt.bfloat16

    from concourse import bacc
    nc = bacc.Bacc("TRN2", target_bir_lowering=False)

    pdram = nc.dram_tensor("pdram", [N_BLK * 128, ROWCOL], bf16,
                           kind="ExternalInput")
    convw = nc.dram_tensor("convw", [128, NF], bf16, kind="ExternalInput")
    hw1w = nc.dram_tensor("hw1w", [128, 3840], bf16, kind="ExternalInput")
    hw2w = nc.dram_tensor("hw2w", [128, 3840], bf16, kind="ExternalInput")
    projw = nc.dram_tensor("projw", [128, 3072], bf16, kind="ExternalInput")
    biases = nc.dram_tensor("biases", [128, 26], f32, kind="ExternalInput")
    out = nc.dram_tensor("out", [D, W], f32, kind="ExternalOutput")

    AluOp = mybir.AluOpType
    ActF = mybir.ActivationFunctionType
    Axis = mybir.AxisListType

    SBLK = 512                   # words per fused superblock
    N_SBLK = W // SBLK           # 8

    with TileContext(nc) as tc, ExitStack() as ctx:
        # ---- persistent tiles ----
        cpool = ctx.enter_context(tc.tile_pool(name="consts", bufs=1))
        wconv_sb = cpool.tile([128, NF], bf16, name="wconv_sb")
        whw1_sb = cpool.tile([128, 3840], bf16, name="whw1_sb")
        whw2_sb = cpool.tile([128, 3840], bf16, name="whw2_sb")
        wproj_sb = cpool.tile([128, 3072], bf16, name="wproj_sb")
        bias_sb = cpool.tile([128, 26], f32, name="bias_sb")
        nc.sync.dma_start(out=wconv_sb[:], in_=convw[:])
        nc.sync.dma_start(out=whw1_sb[:], in_=hw1w[:])
        nc.sync.dma_start(out=whw2_sb[:], in_=hw2w[:])
        nc.sync.dma_start(out=wproj_sb[:], in_=projw[:])
        nc.sync.dma_start(out=bias_sb[:], in_=biases[:])

        conv_lhst = {i: wconv_sb[0:128, CH_OFF[i]:CH_OFF[i] + CH_SZ[i]]
                     for i in range(4)}
        hw_lhst = {}
        for hwi, wsb in ((1, whw1_sb), (2, whw2_sb)):
            c = 0
            for which in ("nl", "g"):
                for i in range(4):
                    for kc in range(4):
                        hw_lhst[(hwi, which, i, kc)] = wsb[0:HW_SZ[kc], c:c + HW_SZ[i]]
                        c += HW_SZ[i]
        proj_lhst = {}
        c = 0
        for mi, (mo, ms) in enumerate(PROJ_M):
            for kc in range(4):
                proj_lhst[(mi, kc)] = wproj_sb[0:HW_SZ[kc], c:c + ms]
                c += ms

        # ---- fused conv+pool+highway+proj, per 512-word superblock ----
        with tc.tile_pool(name="patch", bufs=2) as ppool, \
             tc.tile_pool(name="cpsum", bufs=2, space="PSUM") as cpsum, \
             tc.tile_pool(name="spool", bufs=2) as spool, \
             tc.tile_pool(name="tpool", bufs=2) as tpool, \
             tc.tile_pool(name="upool", bufs=1) as upool, \
             tc.tile_pool(name="hpool", bufs=2) as hpool, \
             tc.tile_pool(name="htmp", bufs=3) as htmp, \
             tc.tile_pool(name="osb", bufs=2) as osb:

            for sblk in range(N_SBLK):
                # h tiles for this superblock: [128, 4*SBLK], chunk i at
                # columns [i*SBLK, (i+1)*SBLK)
                h1 = hpool.tile([128, 4 * SBLK], bf16, tag="h1")
                ptiles = []
                for half in range(2):
                    blk = sblk * 2 + half
                    ptile = ppool.tile([128, ROWCOL], bf16, tag="patch")
                    nc.sync.dma_start(
                        out=ptile[:],
                        in_=pdram[blk * 128:(blk + 1) * 128, :])
                    ptiles.append(ptile)

                # ---- conv + pool -> h1 ----
                for half in range(2):
                    ptile = ptiles[half]
                    for wb in range(BLK // WBLK):
                        wcol = wb * WBLK
                        # per-wb merged staging across the 4 chunks
                        uall = tpool.tile([128, 8192], bf16, tag="uall")
                        s4a = spool.tile([128, 1024], bf16, tag="s4a")
                        ra = spool.tile([128, 512], bf16, tag="ra")
                        for i in range(4):
                            CH = CH_SZ[i]
                            pv = ptile[:]

                            def mk_rhs(slot0, nslots):
                                return bass.AP(
                                    tensor=pv.tensor,
                                    offset=pv.offset + slot0 * BLK + wcol,
                                    ap=[list(pv.ap[0]), [BLK, nslots],
                                        [1, WBLK]],
                                )

                            # T1: slots 0..15 (DVE direct reduce)
                            t1 = cpsum.tile([128, 2048], f32, tag="st",
                                            space="PSUM")
                            for g in range(4):
                                nc.tensor.matmul(
                                    out=t1[0:CH, g * 512:(g + 1) * 512],
                                    lhsT=conv_lhst[i], rhs=mk_rhs(4 * g, 4),
                                    start=True, stop=True)
                            # T2: slots 16..31 (ACT copy)
                            t2 = cpsum.tile([128, 2048], f32, tag="st",
                                            space="PSUM")
                            for g in range(4):
                                nc.tensor.matmul(
                                    out=t2[0:CH, g * 512:(g + 1) * 512],
                                    lhsT=conv_lhst[i], rhs=mk_rhs(16 + 4 * g, 4),
                                    start=True, stop=True)
                            # T3: slots 32..47 (ACT copy)
                            t3 = cpsum.tile([128, 2048], f32, tag="st",
                                            space="PSUM")
                            for g in range(4):
                                nc.tensor.matmul(
                                    out=t3[0:CH, g * 512:(g + 1) * 512],
                                    lhsT=conv_lhst[i], rhs=mk_rhs(32 + 4 * g, 4),
                                    start=True, stop=True)
                            # T4: slots 48..49 (ACT copy)
                            t4 = cpsum.tile([128, 2048], f32, tag="st",
                                            space="PSUM")
                            nc.tensor.matmul(
                                out=t4[0:CH, 0:256],
                                lhsT=conv_lhst[i], rhs=mk_rhs(48, 2),
                                start=True, stop=True)
                            # PE keep-warm fillers into unused banks of t4.
                            # The HAM clock gate halves the PE clock whenever
                            # the PE idles; consumers (DVE/ACT pooling) pace
                            # this phase, so burn the idle on dummy matmuls.
                            for f in range(F_CONV):
                                nc.tensor.matmul(
                                    out=t4[0:CH, 512:1024],
                                    lhsT=conv_lhst[i], rhs=mk_rhs(0, 4),
                                    start=True, stop=True)

                            # DVE: direct reduce T1 -> ra slice
                            t1v = t1[:]
                            nc.vector.tensor_reduce(
                                out=ra[0:CH, i * 128:(i + 1) * 128],
                                in_=bass.AP(tensor=t1v.tensor, offset=t1v.offset,
                                            ap=[[t1v.ap[0][0], CH], [1, WBLK],
                                                [WBLK, 16]]),
                                axis=Axis.X, op=AluOp.max)

                            # ACT: copy T2, T3, T4 -> SBUF bf16
                            s2 = tpool.tile([128, 2048], bf16, tag="s2")
                            nc.scalar.activation(out=s2[0:CH, :],
                                                 in_=t2[0:CH, :],
                                                 func=ActF.Identity, scale=1.0)
                            s3 = tpool.tile([128, 2048], bf16, tag="s3")
                            nc.scalar.activation(out=s3[0:CH, :],
                                                 in_=t3[0:CH, :],
                                                 func=ActF.Identity, scale=1.0)
                            nc.scalar.activation(
                                out=s4a[0:CH, i * 256:(i + 1) * 256],
                                in_=t4[0:CH, 0:256],
                                func=ActF.Identity, scale=1.0)

                            # fold 32 slots -> 16 into the merged tile
                            nc.vector.tensor_tensor(
                                out=uall[0:CH, i * 2048:(i + 1) * 2048],
                                in0=s2[0:CH, :], in1=s3[0:CH, :], op=AluOp.max)

                        # merged tree over all 4 chunks: 16 -> 8 -> 4 -> 2 -> 1
                        uv = uall[:]
                        upitch = uv.ap[0][0]

                        def strided(tile_ap, blk_stride, width, off):
                            return bass.AP(
                                tensor=tile_ap.tensor,
                                offset=tile_ap.offset + off,
                                ap=[[tile_ap.ap[0][0], 128], [blk_stride, 4],
                                    [1, width]])

                        u2 = upool.tile([128, 4096], bf16, tag="u2")
                        nc.vector.tensor_tensor(
                            out=u2[:], in0=strided(uv, 2048, 1024, 0),
                            in1=strided(uv, 2048, 1024, 1024), op=AluOp.max)
                        u2v = u2[:]
                        u3 = spool.tile([128, 2048], bf16, tag="u3")
                        nc.vector.tensor_tensor(
                            out=u3[:], in0=strided(u2v, 1024, 512, 0),
                            in1=strided(u2v, 1024, 512, 512), op=AluOp.max)
                        u3v = u3[:]
                        u4 = spool.tile([128, 1024], bf16, tag="u4")
                        nc.vector.tensor_tensor(
                            out=u4[:], in0=strided(u3v, 512, 256, 0),
                            in1=strided(u3v, 512, 256, 256), op=AluOp.max)
                        u4v = u4[:]
                        u5 = spool.tile([128, 512], bf16, tag="u5")
                        nc.vector.tensor_tensor(
                            out=u5[:], in0=strided(u4v, 256, 128, 0),
                            in1=strided(u4v, 256, 128, 128), op=AluOp.max)
                        s4v = s4a[:]
                        s4f = spool.tile([128, 512], bf16, tag="s4f")
                        nc.vector.tensor_tensor(
                            out=s4f[:], in0=strided(s4v, 256, 128, 0),
                            in1=strided(s4v, 256, 128, 128), op=AluOp.max)
                        vf = spool.tile([128, 512], bf16, tag="vf")
                        nc.vector.tensor_tensor(out=vf[:], in0=u5[:],
                                                in1=s4f[:], op=AluOp.max)
                        # final fold with ra, strided write into h1
                        h1v = h1[:]
                        nc.vector.tensor_tensor(
                            out=bass.AP(tensor=h1v.tensor,
                                        offset=h1v.offset + half * BLK + wcol,
                                        ap=[[h1v.ap[0][0], 128], [SBLK, 4],
                                            [1, WBLK]]),
                            in0=vf[:], in1=ra[:], op=AluOp.max)

                # ---- conv bias (DVE) + leaky on h1 ----
                for i in range(4):
                    CH = CH_SZ[i]
                    hv = h1[0:CH, i * SBLK:(i + 1) * SBLK]
                    nc.vector.tensor_scalar_add(
                        out=hv, in0=hv,
                        scalar1=bias_sb[0:CH, BIAS_CONV + i:BIAS_CONV + i + 1])
                    nc.vector.scalar_tensor_tensor(
                        out=hv, in0=hv, scalar=0.01, in1=hv,
                        op0=AluOp.mult, op1=AluOp.max)

                # ---- highway layers on this superblock ----
                h_cur = h1
                for hwi in (1, 2):
                    h_nxt = hpool.tile([128, 4 * SBLK], bf16, tag=f"h{hwi+1}")
                    for i in range(4):
                        CH = CH_SZ[i]
                        # nonlin
                        ps = cpsum.tile([128, 2048], f32, tag="st",
                                        space="PSUM")
                        for kc in range(4):
                            nc.tensor.matmul(
                                out=ps[0:CH, 0:WT],
                                lhsT=hw_lhst[(hwi, "nl", i, kc)],
                                rhs=h_cur[0:HW_SZ[kc],
                                          kc * SBLK:kc * SBLK + WT],
                                start=(kc == 0), stop=(kc == 3),
                            )
                        nl = htmp.tile([128, WT], bf16, tag="nl")
                        bcol = BIAS_HW[hwi]["nl"] + i
                        nc.scalar.activation(
                            out=nl[0:CH, :], in_=ps[0:CH, 0:WT],
                            func=ActF.Identity,
                            bias=bias_sb[0:CH, bcol:bcol + 1], scale=1.0)
                        nc.vector.scalar_tensor_tensor(
                            out=nl[0:CH, :], in0=nl[0:CH, :], scalar=0.01,
                            in1=nl[0:CH, :], op0=AluOp.mult, op1=AluOp.max)
                        # gate
                        ps2 = cpsum.tile([128, 2048], f32, tag="st",
                                         space="PSUM")
                        for kc in range(4):
                            nc.tensor.matmul(
                                out=ps2[0:CH, 0:WT],
                                lhsT=hw_lhst[(hwi, "g", i, kc)],
                                rhs=h_cur[0:HW_SZ[kc],
                                          kc * SBLK:kc * SBLK + WT],
                                start=(kc == 0), stop=(kc == 3),
                            )
                        for f in range(F_HW):
                            nc.tensor.matmul(
                                out=ps2[0:CH, 512:1024],
                                lhsT=hw_lhst[(hwi, "g", i, 3)],
                                rhs=h_cur[0:128, 3 * SBLK:3 * SBLK + WT],
                                start=True, stop=True)
                        g = htmp.tile([128, WT], bf16, tag="g")
                        bcol = BIAS_HW[hwi]["g"] + i
                        nc.scalar.activation(
                            out=g[0:CH, :], in_=ps2[0:CH, 0:WT],
                            func=ActF.Sigmoid,
                            bias=bias_sb[0:CH, bcol:bcol + 1], scale=1.0)
                        # h' = nl + g*(h - nl)  (gpsimd: sub, mult; dve: add)
                        dtl = htmp.tile([128, WT], bf16, tag="dtl")
                        nc.gpsimd.tensor_tensor(
                            out=dtl[0:CH, :],
                            in0=h_cur[0:CH, i * SBLK:i * SBLK + WT],
                            in1=nl[0:CH, :], op=AluOp.subtract)
                        ug = htmp.tile([128, WT], bf16, tag="ug")
                        nc.gpsimd.tensor_tensor(
                            out=ug[0:CH, :], in0=g[0:CH, :], in1=dtl[0:CH, :],
                            op=AluOp.mult)
                        nc.vector.tensor_tensor(
                            out=h_nxt[0:CH, i * SBLK:i * SBLK + WT],
                            in0=nl[0:CH, :], in1=ug[0:CH, :], op=AluOp.add)
                    h_cur = h_nxt

                # ---- projection ----
                ws = sblk * SBLK
                for mi, (mo, ms) in enumerate(PROJ_M):
                    ps = cpsum.tile([128, 2048], f32, tag="st", space="PSUM")
                    for kc in range(4):
                        nc.tensor.matmul(
                            out=ps[0:ms, 0:WT],
                            lhsT=proj_lhst[(mi, kc)],
                            rhs=h_cur[0:HW_SZ[kc], kc * SBLK:kc * SBLK + WT],
                            start=(kc == 0), stop=(kc == 3),
                        )
                    ot = osb.tile([128, WT], f32, tag="ot")
                    bcol = BIAS_PROJ + mi
                    nc.scalar.activation(
                        out=ot[0:ms, :], in_=ps[0:ms, 0:WT], func=ActF.Identity,
                        bias=bias_sb[0:ms, bcol:bcol + 1], scale=1.0)
                    nc.sync.dma_start(
                        out=out[mo:mo + ms, ws:ws + WT], in_=ot[0:ms, :])

    nc.finalize()
    return nc


# ---------------- host wrapper ----------------

def _prep_shared(inputs):
    return {
        "convw": _prep_conv_lhst(inputs),
        "hw1w": _prep_hw_lhst(inputs["hw1_w"]),
        "hw2w": _prep_hw_lhst(inputs["hw2_w"]),
        "projw": _prep_proj_lhst(inputs["proj_w"]),
        "biases": _prep_biases(inputs),
    }


def _prep_in_maps(inputs):
    X = np.asarray(inputs["X_scan"]).reshape(W_TOTAL, MAX_CHARS).astype(np.int32)
    emb_pad = np.zeros((258, CHAR_D), np.float32)
    emb_pad[:257] = np.asarray(inputs["emb"], np.float32)
    emb_bf = emb_pad.astype(BF16)

    shared = _prep_shared(inputs)
    in_maps = []
    for c in range(N_CORES):
        m = dict(shared)
        m["pdram"] = _prep_pdram(X[c * W:(c + 1) * W], emb_bf)
        in_maps.append(m)
    return in_maps


def _gather_out(res):
    outs = [r["out"] for r in res.results]  # each [768, W]
    full = np.concatenate([o.T for o in outs], axis=0)  # [32768, 768]
    return full.reshape(B, T, D).astype(np.float32)


def kernel(**inputs):
    from concourse.bass_utils import run_bass_kernel_spmd

    nc = _build(W)
    in_maps = _prep_in_maps(inputs)
    res = run_bass_kernel_spmd(nc, in_maps, list(range(N_CORES)))
    return _gather_out(res)
